# revision 6
# baseline (speedup 1.0000x reference)
"""Distributed Trainium2 Bass kernel for the fused attention layer.

Problem (hardcoded):
    B=2, S=2048, D=1024, H=16, HD=64.
    out = softmax((q@Wq+bq) @ (k@Wk+bk)^T / sqrt(HD)) @ (v@Wv+bv), per (b, h).

Sharding: 8 cores, core c -> batch b = c//4, head group hg = c%4 (4 heads).
Each core computes its 4 heads fully independently (no collectives) and
writes a transposed [256, S] bf16 slab; the host reassembles [B, S, D].

Per-core dataflow (matmul inputs bf16, PSUM accumulation f32):
  QhatT [2*128, S]  = (Wq_cols^T @ q_b^T) + bq   (heads on partitions)
  KhatT [2*128, S]  = (Wk_cols^T @ k_b^T) + bk
  Vaug  [S, 4*65]   = (v_b @ Wv_cols) + bv, with a ones column per head
  per head h:
    E[k, q]    = exp((KhatT_h-slice^T x QhatT_h) / 8)   (scores transposed,
                 ScalarE exp straight from PSUM, no max subtraction)
    U[65, q]   = sum_kt Vaug_h[kt]^T @ E[kt]  (row 64 = softmax denominator)
    outT[d, q] = U[0:64, q] * (1/U[64, q])    (denom broadcast via DMA)
Heads are software-pipelined: attn-V of head h-1 interleaves with
scores+exp of head h so ScalarE (the exp bottleneck) never idles.
"""

import sys
import os

for _p in ("/opt/trn_rl_repo",):
    if os.path.isdir(_p) and _p not in sys.path:
        sys.path.append(_p)

import numpy as np
import ml_dtypes

import concourse.bacc as bacc
import concourse.mybir as mybir
from concourse import tile
from concourse.bass_utils import run_bass_kernel_spmd

BF16 = ml_dtypes.bfloat16
N_CORES = 8
B, S, D, H = 2, 2048, 1024, 16
HD = D // H            # 64
HL = 4                 # local heads per core
DH = HL * HD           # 256 local out channels
KT = D // 128          # 8 contraction tiles for projections
ST = S // 128          # 16 sequence tiles

_CACHED = {}


def _build():
    f32 = mybir.dt.float32
    bf = mybir.dt.bfloat16
    EXP = mybir.ActivationFunctionType.Exp
    nc = bacc.Bacc(
        "TRN2", target_bir_lowering=False, debug=False, num_devices=N_CORES
    )

    qT_d = nc.dram_tensor("qT", [D, S], bf, kind="ExternalInput")
    kT_d = nc.dram_tensor("kT", [D, S], bf, kind="ExternalInput")
    vT_d = nc.dram_tensor("vT", [D, S], bf, kind="ExternalInput")
    wq_d = nc.dram_tensor("wq", [D, DH], bf, kind="ExternalInput")
    wk_d = nc.dram_tensor("wk", [D, DH], bf, kind="ExternalInput")
    wv_d = nc.dram_tensor("wv", [D, DH], bf, kind="ExternalInput")
    bq_d = nc.dram_tensor("bq", [128, 2], f32, kind="ExternalInput")
    bk_d = nc.dram_tensor("bk", [128, 2], f32, kind="ExternalInput")
    bv_d = nc.dram_tensor("bv", [128, DH], f32, kind="ExternalInput")
    out_d = nc.dram_tensor("out", [DH, S], bf, kind="ExternalOutput")

    with tile.TileContext(nc) as tc:
        with (
            tc.tile_pool(name="big", bufs=26) as big,
            tc.tile_pool(name="wp", bufs=1) as wp,
            tc.tile_pool(name="qk", bufs=1) as qk,
            tc.tile_pool(name="vp", bufs=1) as vp,
            tc.tile_pool(name="np_", bufs=1) as npl,
            tc.tile_pool(name="outp", bufs=2) as outp,
            tc.tile_pool(name="ps", bufs=2, space="PSUM") as ps,
            tc.tile_pool(name="aps", bufs=1, space="PSUM") as aps,
        ):
            # ---- loads (order matters: Q-projection deps first) ----
            def load_rows(dram, n_tiles, width, pool, eng):
                ts = []
                for i in range(n_tiles):
                    t = (pool.tile([128, width], bf, tag="big",
                                   name=f"{dram.name}t{i}")
                         if pool is big else
                         pool.tile([128, width], bf, tag=f"{dram.name}{i}",
                                   name=f"{dram.name}t{i}"))
                    eng.dma_start(t[:], dram[i * 128:(i + 1) * 128, :])
                    ts.append(t)
                return ts

            wq_t = load_rows(wq_d, KT, DH, wp, nc.gpsimd)
            bq_t = wp.tile([128, 2], f32, tag="bq", name="bq_t")
            nc.gpsimd.dma_start(bq_t[:], bq_d[:])
            qT_t = load_rows(qT_d, KT, S, big, nc.sync)
            wk_t = load_rows(wk_d, KT, DH, wp, nc.gpsimd)
            bk_t = wp.tile([128, 2], f32, tag="bk", name="bk_t")
            nc.gpsimd.dma_start(bk_t[:], bk_d[:])
            kT_t = load_rows(kT_d, KT, S, big, nc.sync)
            wv_t = load_rows(wv_d, KT, DH, wp, nc.gpsimd)
            bv_t = wp.tile([128, DH], f32, tag="bv", name="bv_t")
            nc.gpsimd.dma_start(bv_t[:], bv_d[:])
            vT_t = load_rows(vT_d, KT, S, big, nc.sync)

            qhat = [qk.tile([128, S], bf, tag=f"qh{mp}", name=f"qhat{mp}")
                    for mp in range(2)]
            khat = [qk.tile([128, S], bf, tag=f"kh{mp}", name=f"khat{mp}")
                    for mp in range(2)]
            vaug = [None] * ST
            texp = {}

            def qk_chain(src_t, w_t, b_t, dst, mp, nch):
                pst = ps.tile([128, 1024], f32, tag="ps", name="proj_ps")
                for kt in range(KT):
                    nc.tensor.matmul(
                        pst[:, 0:512],
                        w_t[kt][:, mp * 128:(mp + 1) * 128],
                        src_t[kt][:, nch * 512:(nch + 1) * 512],
                        start=(kt == 0), stop=(kt == KT - 1),
                    )
                nc.vector.tensor_scalar_add(
                    dst[mp][:, nch * 512:(nch + 1) * 512],
                    pst[:, 0:512], b_t[:, mp:mp + 1],
                )

            def v_chain(st):
                pst = ps.tile([128, 1024], f32, tag="ps", name="vproj_ps")
                for kt in range(KT):
                    nc.tensor.matmul(
                        pst[:, 0:DH],
                        vT_t[kt][:, st * 128:(st + 1) * 128],
                        wv_t[kt][:],
                        start=(kt == 0), stop=(kt == KT - 1),
                    )
                va = vp.tile([128, HL * 65], bf, tag=f"va{st}",
                             name=f"vaug{st}")
                nc.vector.memset(va[:], 1.0)
                for h in range(HL):
                    nc.vector.tensor_add(
                        va[:, h * 65:h * 65 + 64],
                        pst[:, h * 64:(h + 1) * 64],
                        bv_t[:, h * 64:(h + 1) * 64],
                    )
                vaug[st] = va

            def scores_step(h, kt):
                mp, hb = h // 2, (h % 2) * 64
                tx = big.tile([128, S], bf, tag="big", name=f"texp_{h}_{kt}")
                for half in range(2):
                    spt = ps.tile([128, 1024], f32, tag="ps", name="score_ps")
                    for qch in range(2):
                        qlo = half * 1024 + qch * 512
                        nc.tensor.matmul(
                            spt[:, qch * 512:(qch + 1) * 512],
                            khat[mp][hb:hb + 64, kt * 128:(kt + 1) * 128],
                            qhat[mp][hb:hb + 64, qlo:qlo + 512],
                            start=True, stop=True,
                        )
                    nc.scalar.activation(
                        tx[:, half * 1024:(half + 1) * 1024], spt[:],
                        EXP, scale=0.125,
                    )
                texp[(h, kt)] = tx

            def attnv_step(h, kt, apt):
                for nch in range(4):
                    nc.tensor.matmul(
                        apt[nch][:],
                        vaug[kt][:, h * 65:h * 65 + 65],
                        texp[(h, kt)][:, nch * 512:(nch + 1) * 512],
                        start=(kt == 0), stop=(kt == ST - 1),
                    )

            def norm(h, apt):
                rec = npl.tile([128, S], f32, tag="rec", name=f"rec{h}")
                for nch in range(4):
                    nc.vector.reciprocal(
                        rec[64:65, nch * 512:(nch + 1) * 512],
                        apt[nch][64:65, :],
                    )
                rec0 = npl.tile([1, S], f32, tag="rec0", name=f"rec0_{h}")
                nc.sync.dma_start(rec0[0:1, :], rec[64:65, :])
                bc = npl.tile([64, S], f32, tag="bc", name=f"bc{h}")
                nc.gpsimd.partition_broadcast(bc[:], rec0[0:1, :])
                ot = outp.tile([64, S], bf, tag="ot", name=f"outT{h}")
                for nch in range(4):
                    nc.vector.tensor_mul(
                        ot[:, nch * 512:(nch + 1) * 512],
                        apt[nch][0:64, :],
                        bc[:, nch * 512:(nch + 1) * 512],
                    )
                nc.sync.dma_start(out_d[h * 64:(h + 1) * 64, :], ot[:])

            # ---- schedule ----
            for nch in range(4):
                qk_chain(qT_t, wq_t, bq_t, qhat, 0, nch)
            for nch in range(4):
                qk_chain(kT_t, wk_t, bk_t, khat, 0, nch)

            apt_prev = None
            for h in range(HL):
                apt = [aps.tile([65, 512], f32, tag=f"at{j}",
                                name=f"attn_ps_{h}_{j}") for j in range(4)]
                for kt in range(ST):
                    if apt_prev is not None:
                        attnv_step(h - 1, kt, apt_prev)
                    scores_step(h, kt)
                    if h == 0:
                        if kt < 4:
                            qk_chain(qT_t, wq_t, bq_t, qhat, 1, kt)
                        elif kt < 8:
                            qk_chain(kT_t, wk_t, bk_t, khat, 1, kt - 4)
                        v_chain(kt)
                if apt_prev is not None:
                    norm(h - 1, apt_prev)
                apt_prev = apt
            for kt in range(ST):
                attnv_step(HL - 1, kt, apt_prev)
            norm(HL - 1, apt_prev)

    nc.compile()
    return nc


def _prep_in_maps(q, k, v, Wq, bq, Wk, bk, Wv, bv):
    qT = [np.ascontiguousarray(q[b].T.astype(BF16)) for b in range(B)]
    kT = [np.ascontiguousarray(k[b].T.astype(BF16)) for b in range(B)]
    vT = [np.ascontiguousarray(v[b].T.astype(BF16)) for b in range(B)]
    in_maps = []
    for c in range(N_CORES):
        b, hg = c // 4, c % 4
        cols = slice(hg * DH, (hg + 1) * DH)
        in_maps.append({
            "qT": qT[b],
            "kT": kT[b],
            "vT": vT[b],
            "wq": np.ascontiguousarray(Wq[:, cols].astype(BF16)),
            "wk": np.ascontiguousarray(Wk[:, cols].astype(BF16)),
            "wv": np.ascontiguousarray(Wv[:, cols].astype(BF16)),
            "bq": np.ascontiguousarray(
                bq[cols].astype(np.float32).reshape(2, 128).T),
            "bk": np.ascontiguousarray(
                bk[cols].astype(np.float32).reshape(2, 128).T),
            "bv": np.ascontiguousarray(
                np.tile(bv[cols].astype(np.float32), (128, 1))),
        })
    return in_maps


def kernel(q, k, v, Wq, bq, Wk, bk, Wv, bv, _trace=False, _trace_cores=None):
    q, k, v = (np.asarray(x, np.float32) for x in (q, k, v))
    Wq, Wk, Wv = (np.asarray(x, np.float32) for x in (Wq, Wk, Wv))
    bq, bk, bv = (np.asarray(x, np.float32) for x in (bq, bk, bv))

    if "nc" not in _CACHED:
        _CACHED["nc"] = _build()
    nc = _CACHED["nc"]

    in_maps = _prep_in_maps(q, k, v, Wq, bq, Wk, bk, Wv, bv)
    res = run_bass_kernel_spmd(
        nc, in_maps, core_ids=list(range(N_CORES)),
        trace=_trace, trace_cores=_trace_cores,
    )
    _CACHED["last_result"] = res

    out = np.empty((B, S, D), np.float32)
    for c in range(N_CORES):
        b, hg = c // 4, c % 4
        out[b, :, hg * DH:(hg + 1) * DH] = \
            res.results[c]["out"].T.astype(np.float32)
    return out


# revision 8
# speedup vs baseline: 1.4437x; 1.4437x over previous
"""Distributed Trainium2 Bass kernel for the fused attention layer.

Problem (hardcoded):
    B=2, S=2048, D=1024, H=16, HD=64.
    out = softmax((q@Wq+bq) @ (k@Wk+bk)^T / sqrt(HD)) @ (v@Wv+bv), per (b, h).

Sharding: 8 cores, core c -> batch b = c//4, head group hg = c%4 (4 heads).
Each core computes its 4 heads fully independently (no collectives) and
writes a transposed [256, S] bf16 slab; the host reassembles [B, S, D].

Per-core dataflow (matmul inputs bf16, PSUM accumulation f32):
  QhatT [2*128, S]  = (Wq_cols^T @ q_b^T) + bq   (heads on partitions)
  KhatT [2*128, S]  = (Wk_cols^T @ k_b^T) + bk
  Vaug  [S, 4*65]   = (v_b @ Wv_cols) + bv, with a ones column per head
  per head h:
    E[k, q]    = exp((KhatT_h-slice^T x QhatT_h) / 8)   (scores transposed,
                 ScalarE exp straight from PSUM, no max subtraction)
    U[65, q]   = sum_kt Vaug_h[kt]^T @ E[kt]  (row 64 = softmax denominator)
    outT[d, q] = U[0:64, q] * (1/U[64, q])    (denom broadcast via DMA)
Heads are software-pipelined: attn-V of head h-1 interleaves with
scores+exp of head h so ScalarE (the exp bottleneck) never idles.
"""

import sys
import os

for _p in ("/opt/trn_rl_repo",):
    if os.path.isdir(_p) and _p not in sys.path:
        sys.path.append(_p)

import numpy as np
import ml_dtypes

import concourse.bacc as bacc
import concourse.mybir as mybir
from concourse import tile
from concourse.bass_utils import run_bass_kernel_spmd

BF16 = ml_dtypes.bfloat16
N_CORES = 8
B, S, D, H = 2, 2048, 1024, 16
HD = D // H            # 64
HL = 4                 # local heads per core
DH = HL * HD           # 256 local out channels
KT = D // 128          # 8 contraction tiles for projections
ST = S // 128          # 16 sequence tiles

_CACHED = {}


def _build():
    f32 = mybir.dt.float32
    bf = mybir.dt.bfloat16
    EXP = mybir.ActivationFunctionType.Exp
    nc = bacc.Bacc(
        "TRN2", target_bir_lowering=False, debug=False, num_devices=N_CORES
    )

    qT_d = nc.dram_tensor("qT", [D, S], bf, kind="ExternalInput")
    kT_d = nc.dram_tensor("kT", [D, S], bf, kind="ExternalInput")
    vT_d = nc.dram_tensor("vT", [D, S], bf, kind="ExternalInput")
    wq_d = nc.dram_tensor("wq", [D, DH], bf, kind="ExternalInput")
    wk_d = nc.dram_tensor("wk", [D, DH], bf, kind="ExternalInput")
    wv_d = nc.dram_tensor("wv", [D, DH], bf, kind="ExternalInput")
    bq_d = nc.dram_tensor("bq", [128, 2], f32, kind="ExternalInput")
    bk_d = nc.dram_tensor("bk", [128, 2], f32, kind="ExternalInput")
    bv_d = nc.dram_tensor("bv", [128, DH], f32, kind="ExternalInput")
    out_d = nc.dram_tensor("out", [DH, S], bf, kind="ExternalOutput")

    with tile.TileContext(nc) as tc:
        with (
            tc.tile_pool(name="big", bufs=25) as big,
            tc.tile_pool(name="wp", bufs=1) as wp,
            tc.tile_pool(name="qk", bufs=1) as qk,
            tc.tile_pool(name="vp", bufs=1) as vp,
            tc.tile_pool(name="np_", bufs=1) as npl,
            tc.tile_pool(name="outp", bufs=1) as outp,
            tc.tile_pool(name="ps", bufs=2, space="PSUM") as ps,
            tc.tile_pool(name="aps", bufs=1, space="PSUM") as aps,
        ):
            # ---- loads (order matters: Q-projection deps first) ----
            def load_rows(dram, n_tiles, width, pool, eng):
                ts = []
                for i in range(n_tiles):
                    t = (pool.tile([128, width], bf, tag="big",
                                   name=f"{dram.name}t{i}")
                         if pool is big else
                         pool.tile([128, width], bf, tag=f"{dram.name}{i}",
                                   name=f"{dram.name}t{i}"))
                    eng.dma_start(t[:], dram[i * 128:(i + 1) * 128, :])
                    ts.append(t)
                return ts

            wq_t = load_rows(wq_d, KT, DH, wp, nc.gpsimd)
            bq_t = wp.tile([128, 2], f32, tag="bq", name="bq_t")
            nc.gpsimd.dma_start(bq_t[:], bq_d[:])
            qT_t = load_rows(qT_d, KT, S, big, nc.sync)
            wk_t = load_rows(wk_d, KT, DH, wp, nc.gpsimd)
            bk_t = wp.tile([128, 2], f32, tag="bk", name="bk_t")
            nc.gpsimd.dma_start(bk_t[:], bk_d[:])
            kT_t = load_rows(kT_d, KT, S, big, nc.sync)
            wv_t = load_rows(wv_d, KT, DH, wp, nc.gpsimd)
            bv_t = wp.tile([128, DH], f32, tag="bv", name="bv_t")
            nc.gpsimd.dma_start(bv_t[:], bv_d[:])
            vT_t = load_rows(vT_d, KT, S, big, nc.sync)

            qhat = [qk.tile([128, S], bf, tag=f"qh{mp}", name=f"qhat{mp}")
                    for mp in range(2)]
            khat = [qk.tile([128, S], bf, tag=f"kh{mp}", name=f"khat{mp}")
                    for mp in range(2)]
            vaug = [None] * ST
            texp = {}

            def qk_chain(src_t, w_t, b_t, dst, mp, nch):
                pst = ps.tile([128, 1024], f32, tag="ps", name="proj_ps")
                for kt in range(KT):
                    nc.tensor.matmul(
                        pst[:, 0:512],
                        w_t[kt][:, mp * 128:(mp + 1) * 128],
                        src_t[kt][:, nch * 512:(nch + 1) * 512],
                        start=(kt == 0), stop=(kt == KT - 1),
                    )
                nc.vector.tensor_scalar_add(
                    dst[mp][:, nch * 512:(nch + 1) * 512],
                    pst[:, 0:512], b_t[:, mp:mp + 1],
                )

            def v_chain(st):
                pst = ps.tile([128, 1024], f32, tag="ps", name="vproj_ps")
                for kt in range(KT):
                    nc.tensor.matmul(
                        pst[:, 0:DH],
                        vT_t[kt][:, st * 128:(st + 1) * 128],
                        wv_t[kt][:],
                        start=(kt == 0), stop=(kt == KT - 1),
                    )
                va = vp.tile([128, HL * 65], bf, tag=f"va{st}",
                             name=f"vaug{st}")
                nc.vector.memset(va[:], 1.0)
                for h in range(HL):
                    nc.vector.tensor_add(
                        va[:, h * 65:h * 65 + 64],
                        pst[:, h * 64:(h + 1) * 64],
                        bv_t[:, h * 64:(h + 1) * 64],
                    )
                vaug[st] = va

            def scores_step(h, kt):
                mp, hb = h // 2, (h % 2) * 64
                tx = big.tile([128, S], bf, tag="big", name=f"texp_{h}_{kt}")
                for half in range(2):
                    spt = ps.tile([128, 1024], f32, tag="ps", name="score_ps")
                    for qch in range(2):
                        qlo = half * 1024 + qch * 512
                        nc.tensor.matmul(
                            spt[:, qch * 512:(qch + 1) * 512],
                            khat[mp][hb:hb + 64, kt * 128:(kt + 1) * 128],
                            qhat[mp][hb:hb + 64, qlo:qlo + 512],
                            start=True, stop=True,
                        )
                    nc.scalar.activation(
                        tx[:, half * 1024:(half + 1) * 1024], spt[:],
                        EXP, scale=0.125,
                    )
                texp[(h, kt)] = tx

            def attnv_step(h, kt, apt):
                for nch in range(4):
                    nc.tensor.matmul(
                        apt[nch][0:65, :],
                        vaug[kt][:, h * 65:h * 65 + 65],
                        texp[(h, kt)][:, nch * 512:(nch + 1) * 512],
                        start=(kt == 0), stop=(kt == ST - 1),
                    )

            def norm(h, apt):
                usb = []
                for nch in range(4):
                    u = npl.tile([96, 512], f32, tag=f"usb{nch}",
                                 name=f"usb_{h}_{nch}")
                    nc.vector.tensor_copy(u[:], apt[nch][0:96, :])
                    usb.append(u)
                denrow = npl.tile([1, S], f32, tag="denrow", name=f"den{h}")
                for nch in range(4):
                    nc.sync.dma_start(
                        denrow[0:1, nch * 512:(nch + 1) * 512],
                        usb[nch][64:65, :],
                    )
                rec0 = npl.tile([1, S], f32, tag="rec0", name=f"rec0_{h}")
                nc.vector.reciprocal_approx_fast(rec0[0:1, :], denrow[0:1, :])
                bc = npl.tile([64, S], f32, tag="bc", name=f"bc{h}")
                nc.gpsimd.partition_broadcast(bc[:], rec0[0:1, :])
                ot = outp.tile([64, S], bf, tag="ot", name=f"outT{h}")
                for nch in range(4):
                    nc.vector.tensor_mul(
                        ot[:, nch * 512:(nch + 1) * 512],
                        usb[nch][0:64, :],
                        bc[:, nch * 512:(nch + 1) * 512],
                    )
                nc.sync.dma_start(out_d[h * 64:(h + 1) * 64, :], ot[:])

            # ---- schedule ----
            for nch in range(4):
                qk_chain(qT_t, wq_t, bq_t, qhat, 0, nch)
            for nch in range(4):
                qk_chain(kT_t, wk_t, bk_t, khat, 0, nch)

            apt_prev = None
            for h in range(HL):
                apt = [aps.tile([96, 512], f32, tag=f"at{j}",
                                name=f"attn_ps_{h}_{j}") for j in range(4)]
                for kt in range(ST):
                    if apt_prev is not None:
                        attnv_step(h - 1, kt, apt_prev)
                    scores_step(h, kt)
                    if h == 0:
                        if kt < 4:
                            qk_chain(qT_t, wq_t, bq_t, qhat, 1, kt)
                        elif kt < 8:
                            qk_chain(kT_t, wk_t, bk_t, khat, 1, kt - 4)
                        v_chain(kt)
                if apt_prev is not None:
                    norm(h - 1, apt_prev)
                apt_prev = apt
            for kt in range(ST):
                attnv_step(HL - 1, kt, apt_prev)
            norm(HL - 1, apt_prev)

    nc.compile()
    return nc


def _prep_in_maps(q, k, v, Wq, bq, Wk, bk, Wv, bv):
    qT = [np.ascontiguousarray(q[b].T.astype(BF16)) for b in range(B)]
    kT = [np.ascontiguousarray(k[b].T.astype(BF16)) for b in range(B)]
    vT = [np.ascontiguousarray(v[b].T.astype(BF16)) for b in range(B)]
    in_maps = []
    for c in range(N_CORES):
        b, hg = c // 4, c % 4
        cols = slice(hg * DH, (hg + 1) * DH)
        in_maps.append({
            "qT": qT[b],
            "kT": kT[b],
            "vT": vT[b],
            "wq": np.ascontiguousarray(Wq[:, cols].astype(BF16)),
            "wk": np.ascontiguousarray(Wk[:, cols].astype(BF16)),
            "wv": np.ascontiguousarray(Wv[:, cols].astype(BF16)),
            "bq": np.ascontiguousarray(
                bq[cols].astype(np.float32).reshape(2, 128).T),
            "bk": np.ascontiguousarray(
                bk[cols].astype(np.float32).reshape(2, 128).T),
            "bv": np.ascontiguousarray(
                np.tile(bv[cols].astype(np.float32), (128, 1))),
        })
    return in_maps


def kernel(q, k, v, Wq, bq, Wk, bk, Wv, bv, _trace=False, _trace_cores=None):
    q, k, v = (np.asarray(x, np.float32) for x in (q, k, v))
    Wq, Wk, Wv = (np.asarray(x, np.float32) for x in (Wq, Wk, Wv))
    bq, bk, bv = (np.asarray(x, np.float32) for x in (bq, bk, bv))

    if "nc" not in _CACHED:
        _CACHED["nc"] = _build()
    nc = _CACHED["nc"]

    in_maps = _prep_in_maps(q, k, v, Wq, bq, Wk, bk, Wv, bv)
    res = run_bass_kernel_spmd(
        nc, in_maps, core_ids=list(range(N_CORES)),
        trace=_trace, trace_cores=_trace_cores,
    )
    _CACHED["last_result"] = res

    out = np.empty((B, S, D), np.float32)
    for c in range(N_CORES):
        b, hg = c // 4, c % 4
        out[b, :, hg * DH:(hg + 1) * DH] = \
            res.results[c]["out"].T.astype(np.float32)
    return out
